# revision 5
# baseline (speedup 1.0000x reference)
"""Trainium2 Bass kernel for a classical LSTM (T=512, B=128, D=512, H=512).

Strategy: data-parallel over batch (8 cores x 16 batch each), weights
replicated. Phase 1 computes x_proj = x @ W_x + bias for all timesteps
(fully parallel). Phase 2 runs the sequential recurrence; per step the
gate PSUM is seeded with x_proj via an identity matmul and h @ W_h is
accumulated on top, followed by sigmoid/tanh activations and the cell
update on the vector/scalar engines.
"""

import sys

sys.path.insert(0, "/opt/trn_rl_repo")

import numpy as np

import concourse.bass as bass  # noqa: F401
import concourse.mybir as mybir
import concourse.tile as tile
from concourse import bacc
from concourse.bass_utils import run_bass_kernel_spmd

T, B, D, H = 512, 128, 512, 512
G = 4 * H  # 2048 fused gate width
N_CORES = 8
BC = B // N_CORES  # 16 batch rows per core
FP32 = mybir.dt.float32

_CACHE: dict = {}


def _build(t_steps: int = T):
    nc = bacc.Bacc("TRN2", target_bir_lowering=False, debug=False)

    x_d = nc.dram_tensor("x", [t_steps, BC, D], FP32, kind="ExternalInput")
    wx_d = nc.dram_tensor("wx", [D, G], FP32, kind="ExternalInput")
    wh_d = nc.dram_tensor("wh", [H, G], FP32, kind="ExternalInput")
    bias_d = nc.dram_tensor("bias", [1, G], FP32, kind="ExternalInput")
    id16_d = nc.dram_tensor("id16", [16, 16], FP32, kind="ExternalInput")
    ones_d = nc.dram_tensor("ones", [1, 128], FP32, kind="ExternalInput")
    y_d = nc.dram_tensor("y", [t_steps, BC, H], FP32, kind="ExternalOutput")
    c_d = nc.dram_tensor("c_out", [BC, H], FP32, kind="ExternalOutput")

    x_flat = x_d.rearrange("t b d -> (t b) d")
    n_tb = (t_steps * BC) // 128  # tb-chunks of 128 rows in phase 1

    KD = D // 128  # 4 contraction chunks for x part
    KH = H // 128  # 4 contraction chunks for h part
    NG = G // 512  # 4 gate n-chunks (f, i, g, o)

    with tile.TileContext(nc) as tc:
        with (
            tc.tile_pool(name="const", bufs=1) as constp,
            tc.tile_pool(name="weights", bufs=1) as wpool,
            tc.tile_pool(name="state", bufs=1) as statep,
            tc.tile_pool(name="xin", bufs=3) as xinp,
            tc.tile_pool(name="xt", bufs=3) as xtp,
            tc.tile_pool(name="xpsb", bufs=8) as xpsbp,
            tc.tile_pool(name="gact", bufs=2) as gactp,
            tc.tile_pool(name="dram", bufs=1, space="DRAM") as dramp,
        ):
            # ---- constants / weights resident in SBUF ----
            id16 = constp.tile([16, 16], FP32)
            nc.sync.dma_start(id16[:], id16_d[:])
            ones1 = constp.tile([1, 128], FP32)
            nc.sync.dma_start(ones1[:], ones_d[:])
            bias_sb = constp.tile([1, G], FP32)
            nc.sync.dma_start(bias_sb[:], bias_d[:])
            id128 = constp.tile([128, 128], FP32)
            nc.gpsimd.memset(id128[:], 0.0)
            from concourse.masks import make_identity

            make_identity(nc, id128[:], nomemset=True)

            wx_sb = []
            wh_sb = []
            for k in range(KD):
                w = wpool.tile([128, G], FP32, tag=f"wx{k}")
                nc.sync.dma_start(w[:], wx_d[128 * k : 128 * (k + 1), :])
                wx_sb.append(w)
            for k in range(KH):
                w = wpool.tile([128, G], FP32, tag=f"wh{k}")
                nc.sync.dma_start(w[:], wh_d[128 * k : 128 * (k + 1), :])
                wh_sb.append(w)

            xp_dram = dramp.tile([t_steps * BC, G], FP32)

            # ---- phase 1: x_proj = x @ W_x + bias (all timesteps) ----
            with (
                tc.tile_pool(name="psum1t", bufs=2, space="PSUM") as psumt1,
                tc.tile_pool(name="psum1g", bufs=4, space="PSUM") as psum1,
            ):
                for j in range(n_tb):
                    xin = xinp.tile([128, D], FP32)
                    nc.sync.dma_start(xin[:], x_flat[128 * j : 128 * (j + 1), :])
                    # transpose to x_T: psum tile [128, 512], chunk k at [:, 128k:]
                    pt = psumt1.tile([128, D], FP32, tag="p1t")
                    for k in range(KD):
                        nc.tensor.transpose(
                            pt[:, 128 * k : 128 * (k + 1)],
                            xin[:, 128 * k : 128 * (k + 1)],
                            id128[:],
                        )
                    xt = xtp.tile([128, D], FP32)
                    nc.vector.tensor_copy(xt[:], pt[:])

                    for n in range(NG):
                        pg = psum1.tile([128, 512], FP32, tag="p1g")
                        # bias row via ones: start the accumulation group
                        nc.tensor.matmul(
                            pg[:],
                            ones1[:],
                            bias_sb[:, 512 * n : 512 * (n + 1)],
                            start=True,
                            stop=False,
                        )
                        for k in range(KD):
                            nc.tensor.matmul(
                                pg[:],
                                xt[:, 128 * k : 128 * (k + 1)],
                                wx_sb[k][:, 512 * n : 512 * (n + 1)],
                                start=False,
                                stop=(k == KD - 1),
                            )
                        go = gactp.tile([128, 512], FP32, tag="p1o")
                        if n % 2 == 0:
                            nc.vector.tensor_copy(go[:], pg[:])
                        else:
                            nc.scalar.copy(go[:], pg[:])
                        nc.sync.dma_start(
                            xp_dram[128 * j : 128 * (j + 1), 512 * n : 512 * (n + 1)],
                            go[:],
                        )

            # ---- phase 2: recurrence ----
            with (
                tc.tile_pool(name="psum2g", bufs=1, space="PSUM") as psum2,
                tc.tile_pool(name="psum2t", bufs=2, space="PSUM") as psumt2,
            ):
                c_sb = statep.tile([BC, H], FP32)
                h_sb = statep.tile([BC, H], FP32)
                hT_sb = statep.tile([128, KH * BC], FP32)
                nc.gpsimd.memset(c_sb[:], 0.0)
                nc.gpsimd.memset(hT_sb[:], 0.0)

                sig = mybir.ActivationFunctionType.Sigmoid
                tanh = mybir.ActivationFunctionType.Tanh

                for t in range(t_steps):
                    xp = xpsbp.tile([BC, G], FP32)
                    nc.sync.dma_start(xp[:], xp_dram[BC * t : BC * (t + 1), :])

                    pgs = []
                    for n in range(NG):
                        pg = psum2.tile([BC, 512], FP32, tag=f"g{n}")
                        pgs.append(pg)
                        # seed with x_proj (identity matmul starts the group)
                        nc.tensor.matmul(
                            pg[:],
                            id16[:],
                            xp[:, 512 * n : 512 * (n + 1)],
                            start=True,
                            stop=False,
                        )
                        for k in range(KH):
                            nc.tensor.matmul(
                                pg[:],
                                hT_sb[:, BC * k : BC * (k + 1)],
                                wh_sb[k][:, 512 * n : 512 * (n + 1)],
                                start=False,
                                stop=(k == KH - 1),
                            )

                    f_s = gactp.tile([BC, H], FP32, tag="f")
                    i_s = gactp.tile([BC, H], FP32, tag="i")
                    g_s = gactp.tile([BC, H], FP32, tag="g")
                    o_s = gactp.tile([BC, H], FP32, tag="o")
                    ct1 = gactp.tile([BC, H], FP32, tag="ct1")
                    ct2 = gactp.tile([BC, H], FP32, tag="ct2")
                    tc_s = gactp.tile([BC, H], FP32, tag="tc")

                    nc.scalar.activation(f_s[:], pgs[0][:], sig)
                    nc.scalar.activation(i_s[:], pgs[1][:], sig)
                    nc.vector.tensor_mul(ct1[:], f_s[:], c_sb[:])
                    nc.scalar.activation(g_s[:], pgs[2][:], tanh)
                    nc.vector.tensor_mul(ct2[:], i_s[:], g_s[:])
                    nc.vector.tensor_add(c_sb[:], ct1[:], ct2[:])
                    nc.scalar.activation(tc_s[:], c_sb[:], tanh)
                    nc.scalar.activation(o_s[:], pgs[3][:], sig)
                    nc.vector.tensor_mul(h_sb[:], o_s[:], tc_s[:])

                    nc.sync.dma_start(y_d[t, :, :], h_sb[:])

                    # transpose h for the next step's stationary operand
                    if t != t_steps - 1:
                        pt = psumt2.tile([128, KH * BC], FP32, tag="ht")
                        for k in range(KH):
                            nc.tensor.transpose(
                                pt[:, BC * k : BC * (k + 1)],
                                h_sb[:, 128 * k : 128 * (k + 1)],
                                id16[:],
                            )
                        nc.vector.tensor_copy(hT_sb[:], pt[:])

                nc.sync.dma_start(c_d[:], c_sb[:])

    nc.finalize()
    return nc


def _get_nc(t_steps: int = T):
    key = ("nc", t_steps)
    if key not in _CACHE:
        _CACHE[key] = _build(t_steps)
    return _CACHE[key]


def kernel(inputs, W_f, b_f, W_i, b_i, W_g, b_g, W_o, b_o, t_steps: int = T):
    W = np.concatenate([W_f, W_i, W_g, W_o], axis=1).astype(np.float32)
    bias = np.concatenate([b_f, b_i, b_g, b_o], axis=0).astype(np.float32)

    wx = np.ascontiguousarray(W[:D])
    wh = np.ascontiguousarray(W[D:])
    bias2 = np.ascontiguousarray(bias[None, :])
    id16 = np.eye(16, dtype=np.float32)
    ones = np.ones((1, 128), dtype=np.float32)

    nc = _get_nc(t_steps)
    in_maps = []
    for c in range(N_CORES):
        xc = np.ascontiguousarray(inputs[:t_steps, BC * c : BC * (c + 1), :], dtype=np.float32)
        in_maps.append(
            {
                "x": xc,
                "wx": wx,
                "wh": wh,
                "bias": bias2,
                "id16": id16,
                "ones": ones,
            }
        )

    res = run_bass_kernel_spmd(nc, in_maps, core_ids=list(range(N_CORES)))

    y = np.concatenate([res.results[c]["y"] for c in range(N_CORES)], axis=1)
    cx = np.concatenate([res.results[c]["c_out"] for c in range(N_CORES)], axis=0)
    hx = y[-1]
    return y, (hx, cx)


# revision 15
# speedup vs baseline: 1.4790x; 1.4790x over previous
"""Trainium2 Bass kernel for a classical LSTM (T=512, B=128, D=512, H=512).

Strategy: data-parallel over batch (8 cores x 16 batch each), weights
replicated. Phase 1 computes x_proj = x @ W_x + bias for all timesteps and
spills it to DRAM; its chunks are interleaved into the recurrence's PE idle
gaps. Phase 2 runs the sequential recurrence; per step the gate PSUM is
seeded with x_proj via an identity matmul, h @ W_h accumulates on top
(k-outer order), sigmoid/tanh run on the scalar engine, the cell update on
the vector engine, and the o-gate tail is chunked into 4 column blocks so
the next step's first matmuls start as soon as the first transposed-h chunk
is ready. All matmuls use float32r (full-rate PE; plain fp32 is 1/4 rate).
"""

import sys

sys.path.insert(0, "/opt/trn_rl_repo")

import numpy as np

import concourse.mybir as mybir
import concourse.tile as tile
from concourse import bacc
from concourse.bass_utils import run_bass_kernel_spmd

T, B, D, H = 512, 128, 512, 512
G = 4 * H  # 2048 fused gate width
N_CORES = 8
BC = B // N_CORES  # 16 batch rows per core
FP32 = mybir.dt.float32
F32R = mybir.dt.float32r

_CACHE: dict = {}


def _build(t_steps: int = T):
    nc = bacc.Bacc("TRN2", target_bir_lowering=False, debug=False)

    x_d = nc.dram_tensor("x", [t_steps, BC, D], FP32, kind="ExternalInput")
    wx_d = nc.dram_tensor("wx", [D, G], F32R, kind="ExternalInput")
    wh_d = nc.dram_tensor("wh", [H, G], F32R, kind="ExternalInput")
    bias_d = nc.dram_tensor("bias", [1, G], F32R, kind="ExternalInput")
    id16_d = nc.dram_tensor("id16", [16, 16], F32R, kind="ExternalInput")
    ones_d = nc.dram_tensor("ones", [1, 128], F32R, kind="ExternalInput")
    y_d = nc.dram_tensor("y", [t_steps, BC, H], FP32, kind="ExternalOutput")
    c_d = nc.dram_tensor("c_out", [BC, H], FP32, kind="ExternalOutput")

    x_flat = x_d.rearrange("t b d -> (t b) d")
    n_tb = (t_steps * BC) // 128  # phase-1 tb-chunks of 128 rows

    KD = D // 128
    KH = H // 128
    NG = G // 512
    sig = mybir.ActivationFunctionType.Sigmoid
    tanh = mybir.ActivationFunctionType.Tanh

    with tile.TileContext(nc) as tc:
        with (
            tc.tile_pool(name="const", bufs=1) as constp,
            tc.tile_pool(name="weights", bufs=1) as wpool,
            tc.tile_pool(name="state", bufs=1) as statep,
            tc.tile_pool(name="xin", bufs=3) as xinp,
            tc.tile_pool(name="xt", bufs=2) as xtp,
            tc.tile_pool(name="xpsb", bufs=6) as xpsbp,
            tc.tile_pool(name="gact", bufs=2) as gactp,
            tc.tile_pool(name="hpool", bufs=3) as hpool,
            tc.tile_pool(name="psA", bufs=1, space="PSUM") as psA,  # f,i,g banks
            tc.tile_pool(name="psO", bufs=2, space="PSUM") as psO,  # o bank x2
            tc.tile_pool(name="psT", bufs=1, space="PSUM") as psT,  # hT transposes
            tc.tile_pool(name="ps1t", bufs=1, space="PSUM") as ps1t,  # phase1 xT
            tc.tile_pool(name="ps1g", bufs=1, space="PSUM") as ps1g,  # phase1 gates
            tc.tile_pool(name="dram", bufs=1, space="DRAM") as dramp,
        ):
            # ---- constants / weights resident in SBUF ----
            id16 = constp.tile([16, 16], F32R)
            nc.sync.dma_start(id16[:], id16_d[:])
            id16_f32 = constp.tile([16, 16], FP32)
            nc.gpsimd.dma_start(id16_f32[:], id16_d[:])
            ones1 = constp.tile([1, 128], F32R)
            nc.sync.dma_start(ones1[:], ones_d[:])
            bias_sb = constp.tile([1, G], F32R)
            nc.sync.dma_start(bias_sb[:], bias_d[:])
            id128 = constp.tile([128, 128], FP32)
            nc.gpsimd.memset(id128[:], 0.0)
            from concourse.masks import make_identity

            make_identity(nc, id128[:], nomemset=True)

            wx_sb = []
            wh_sb = []
            for k in range(KD):
                w = wpool.tile([128, G], F32R, tag=f"wx{k}")
                nc.sync.dma_start(w[:], wx_d[128 * k : 128 * (k + 1), :])
                wx_sb.append(w)
            for k in range(KH):
                w = wpool.tile([128, G], F32R, tag=f"wh{k}")
                nc.sync.dma_start(w[:], wh_d[128 * k : 128 * (k + 1), :])
                wh_sb.append(w)

            xp_dram = dramp.tile([t_steps * BC, G], F32R)

            def phase1_chunk(j):
                """x_proj for tb rows [128j, 128j+128): transpose x, matmul, spill."""
                xin = xinp.tile([128, D], FP32)
                nc.sync.dma_start(xin[:], x_flat[128 * j : 128 * (j + 1), :])
                pt = ps1t.tile([128, D], FP32, tag="p1t")
                for k in range(KD):
                    nc.tensor.transpose(
                        pt[:, 128 * k : 128 * (k + 1)],
                        xin[:, 128 * k : 128 * (k + 1)],
                        id128[:],
                    )
                xt = xtp.tile([128, D], F32R)
                nc.vector.tensor_copy(xt[:], pt[:])

                for n in range(NG):
                    pg = ps1g.tile([128, 512], FP32, tag="p1g")
                    nc.tensor.matmul(
                        pg[:],
                        ones1[:],
                        bias_sb[:, 512 * n : 512 * (n + 1)],
                        start=True,
                        stop=False,
                    )
                    for k in range(KD):
                        nc.tensor.matmul(
                            pg[:],
                            xt[:, 128 * k : 128 * (k + 1)],
                            wx_sb[k][:, 512 * n : 512 * (n + 1)],
                            start=False,
                            stop=(k == KD - 1),
                        )
                    go = gactp.tile([128, 512], F32R, tag="p1o")
                    if n % 2 == 0:
                        nc.vector.tensor_copy(go[:], pg[:])
                    else:
                        nc.scalar.copy(go[:], pg[:])
                    nc.sync.dma_start(
                        xp_dram[128 * j : 128 * (j + 1), 512 * n : 512 * (n + 1)],
                        go[:],
                    )

            # ---- state ----
            c_sb = statep.tile([BC, H], FP32)
            hT_sb = statep.tile([128, KH * BC], F32R)
            nc.gpsimd.memset(c_sb[:], 0.0)
            nc.gpsimd.memset(hT_sb[:].bitcast(FP32), 0.0)

            # phase-1 prologue: enough chunks to cover the recurrence lead
            LEAD = 8  # chunks ahead (64 steps of xp)
            done = min(LEAD, n_tb)
            for j in range(done):
                phase1_chunk(j)

            for t in range(t_steps):
                # interleave one phase-1 chunk every 8 steps, kept ~56 steps ahead
                if t % 8 == 0 and done < n_tb:
                    phase1_chunk(done)
                    done += 1

                xp = xpsbp.tile([BC, G], F32R)
                nc.sync.dma_start(xp[:], xp_dram[BC * t : BC * (t + 1), :])

                pgs = []
                for n in range(NG):
                    pool = psO if n == 3 else psA
                    pg = pool.tile([BC, 512], FP32, tag=f"g{n}" if n < 3 else "go")
                    pgs.append(pg)
                    nc.tensor.matmul(
                        pg[:],
                        id16[:],
                        xp[:, 512 * n : 512 * (n + 1)],
                        start=True,
                        stop=False,
                    )
                # n-outer: each gate's bank completes early (f,i,g staggered, o last)
                for n in range(NG):
                    for k in range(KH):
                        nc.tensor.matmul(
                            pgs[n][:],
                            hT_sb[:, BC * k : BC * (k + 1)],
                            wh_sb[k][:, 512 * n : 512 * (n + 1)],
                            start=False,
                            stop=(k == KH - 1),
                        )

                f_s = gactp.tile([BC, H], FP32, tag="f")
                i_s = gactp.tile([BC, H], FP32, tag="i")
                g_s = gactp.tile([BC, H], FP32, tag="g")
                ct1 = gactp.tile([BC, H], FP32, tag="ct1")
                ct2 = gactp.tile([BC, H], FP32, tag="ct2")
                tc_s = gactp.tile([BC, H], FP32, tag="tc")
                o_s = gactp.tile([BC, H], FP32, tag="o")
                h_sb = hpool.tile([BC, H], FP32, tag="h")

                nc.scalar.activation(f_s[:], pgs[0][:], sig)
                nc.scalar.activation(i_s[:], pgs[1][:], sig)
                nc.gpsimd.tensor_mul(ct1[:], f_s[:], c_sb[:])
                nc.scalar.activation(g_s[:], pgs[2][:], tanh)
                nc.vector.tensor_mul(ct2[:], i_s[:], g_s[:])

                # chunked tail: c -> tanh_c -> sig_o -> h -> transpose -> hT copy
                last = t == t_steps - 1
                pt = None if last else psT.tile([128, KH * BC], FP32, tag="ht")
                for kc in range(KH):
                    cs = slice(128 * kc, 128 * (kc + 1))
                    nc.vector.tensor_add(c_sb[:, cs], ct1[:, cs], ct2[:, cs])
                    nc.scalar.activation(tc_s[:, cs], c_sb[:, cs], tanh)
                    nc.scalar.activation(o_s[:, cs], pgs[3][:, cs], sig)
                    nc.vector.tensor_mul(h_sb[:, cs], o_s[:, cs], tc_s[:, cs])
                    if not last:
                        nc.tensor.transpose(
                            pt[:, BC * kc : BC * (kc + 1)], h_sb[:, cs], id16_f32[:]
                        )
                        nc.scalar.activation(
                            hT_sb[:, BC * kc : BC * (kc + 1)],
                            pt[:, BC * kc : BC * (kc + 1)],
                            mybir.ActivationFunctionType.Copy,
                        )

                nc.sync.dma_start(y_d[t, :, :], h_sb[:])

            nc.sync.dma_start(c_d[:], c_sb[:])

    nc.finalize()
    return nc


def _get_nc(t_steps: int = T):
    key = ("nc", t_steps)
    if key not in _CACHE:
        _CACHE[key] = _build(t_steps)
    return _CACHE[key]


def kernel(inputs, W_f, b_f, W_i, b_i, W_g, b_g, W_o, b_o, t_steps: int = T):
    W = np.concatenate([W_f, W_i, W_g, W_o], axis=1).astype(np.float32)
    bias = np.concatenate([b_f, b_i, b_g, b_o], axis=0).astype(np.float32)

    wx = np.ascontiguousarray(W[:D])
    wh = np.ascontiguousarray(W[D:])
    bias2 = np.ascontiguousarray(bias[None, :])
    id16 = np.eye(16, dtype=np.float32)
    ones = np.ones((1, 128), dtype=np.float32)

    nc = _get_nc(t_steps)
    in_maps = []
    for c in range(N_CORES):
        xc = np.ascontiguousarray(
            inputs[:t_steps, BC * c : BC * (c + 1), :], dtype=np.float32
        )
        in_maps.append(
            {"x": xc, "wx": wx, "wh": wh, "bias": bias2, "id16": id16, "ones": ones}
        )

    res = run_bass_kernel_spmd(nc, in_maps, core_ids=list(range(N_CORES)))

    y = np.concatenate([res.results[c]["y"] for c in range(N_CORES)], axis=1)
    cx = np.concatenate([res.results[c]["c_out"] for c in range(N_CORES)], axis=0)
    hx = y[-1]
    return y, (hx, cx)
